# revision 1
# baseline (speedup 1.0000x reference)
"""Trainium2 Bass kernel for the Haar-mask MLP (histogram_binning).

Key algorithmic fact: every Haar interval edge is a multiple of 2^-10, so the
reference's masks -- and therefore the entire MLP output -- depend only on
u = floor(t * 1024) (1024 possible values, exact in fp32 since *1024 is a
power-of-two scale).  The whole network collapses to a 1024x3 lookup table,
computed once on host from the tiny weights.  The device work is the
memory-bound part: stream t, compute u, gather LUT[u], stream out.

Device plan (pure data parallel, 8 cores, 16384 elements each):
  - DMA t chunk into SBUF [128p x 128] (host pre-permutes so that partition
    16c+p, slot s holds element 2048c + 16s + p -- exactly the wrapped index
    layout the GpSimd gathers want).
  - u = floor(t*1024) on DVE (exact under any convert rounding mode),
    convert+clamp+scale to a 16-bit index.
  - Replicate the LUT per partition; GpSimd gather per chunk; DMA back.

Gather impl variants (GATHER_IMPL):
  ap3: ap_gather, d=3 rows           -- table [1024,3]/partition, out interleaved
  ic3: indirect_copy, inner=3, idx*3 -- same layout, resident HW-assisted op
  ap1: ap_gather, d=1, planar table  -- partition p holds LUT[:, p%16%3]
  ic1: indirect_copy, inner=1, planar
The *3 variants DMA partitions {16c} (rows of 512x3); the *1 variants DMA the
full tile and the host picks rows {16c+f}.
"""

from contextlib import ExitStack

import numpy as np

import concourse.tile as tile
from concourse import bacc, mybir
from concourse.bass_utils import run_bass_kernel_spmd

N_CORES = 8
B, T, F = 16, 8192, 3
N = B * T                    # 131072 total elements
NPC = N // N_CORES           # 16384 per neuron core
P = 128                      # SBUF partitions
S = NPC // P                 # 128 slots per partition
NBINS = 1024
NCHUNK = 4                   # gather/store pipeline chunks
IDXS = NPC // 8 // NCHUNK    # 512 indices per q7-core per chunk

GATHER_IMPL = "ic1"
RUN_KWARGS = {}              # test harness may set {"trace": True}
LAST_RESULTS = None
_CACHE = {}


def _build_lut(W1, b1, W2, b2, W3, b3):
    """MLP output for each of the 1024 half-interval bins, fp32 math."""
    u = np.arange(NBINS)
    acc = np.zeros((NBINS, W1.shape[1]), np.float32)
    for j in range(10):
        k = u >> (10 - j)                       # floor(t * 2^j) for t in bin u
        idx = (1 << j) - 1 + k                  # level-j block offset + k
        sign = np.where((u >> (9 - j)) & 1 == 0, np.float32(1), np.float32(-1))
        acc = acc + sign[:, None] * W1[idx]
    h = np.maximum(acc + b1, np.float32(0))
    h = np.maximum(h @ W2 + b2, np.float32(0))
    return (h @ W3 + b3).astype(np.float32)     # (1024, 3)


def _build_nc(impl):
    planar = impl.endswith("1")
    use_ic = impl.startswith("ic")
    row = NBINS if planar else NBINS * F        # table row elements/partition
    gw = IDXS if planar else IDXS * F           # gather out elements/partition

    nc = bacc.Bacc("TRN2", target_bir_lowering=False, debug=False,
                   enable_asserts=False, num_devices=N_CORES)
    f32 = mybir.dt.float32
    idt = mybir.dt.uint16 if use_ic else mybir.dt.int16
    t_d = nc.dram_tensor("t", [P, S], f32, kind="ExternalInput")
    lut_d = nc.dram_tensor("lut", [P, row], f32, kind="ExternalInput")
    if planar:
        out_d = nc.dram_tensor("out", [NCHUNK, P, IDXS], f32,
                               kind="ExternalOutput")
    else:
        out_d = nc.dram_tensor("out", [8, NCHUNK, IDXS * F], f32,
                               kind="ExternalOutput")

    with tile.TileContext(nc) as tc, ExitStack() as ctx:
        cpool = ctx.enter_context(tc.tile_pool(name="c", bufs=1))
        gpool = ctx.enter_context(tc.tile_pool(name="g", bufs=1))

        t_sb = cpool.tile([P, S], f32)
        nc.sync.dma_start(t_sb[:], t_d[:, :])

        # split the table broadcast across partition quarters AND across
        # engines, so each lands on its own HWDGE queue (the broadcast gates
        # the first gather; same-engine splits would serialize on one queue)
        tab = cpool.tile([P, row], f32)
        for q, eng in enumerate((nc.sync, nc.scalar, nc.sync, nc.scalar)):
            eng.dma_start(tab[q * 32:(q + 1) * 32, :],
                          lut_d[q * 32:(q + 1) * 32, :])

        # exact floor(t*1024): round-to-int (any rounding mode), then
        # subtract 1 wherever the rounded value exceeds the true value
        uf = cpool.tile([P, S], f32)
        ii = cpool.tile([P, S], mybir.dt.int32)
        fb = cpool.tile([P, S], f32)
        adj = cpool.tile([P, S], f32)
        ui = cpool.tile([P, S], f32)
        idx = cpool.tile([P, S], idt)
        nc.vector.tensor_scalar(uf[:], t_sb[:], 1024.0, None,
                                mybir.AluOpType.mult)
        nc.vector.tensor_copy(ii[:], uf[:])
        nc.vector.tensor_copy(fb[:], ii[:])
        nc.vector.tensor_tensor(adj[:], fb[:], uf[:], mybir.AluOpType.is_gt)
        nc.vector.tensor_sub(ui[:], fb[:], adj[:])
        if use_ic and not planar:               # scale idx by 3 for ranges
            mn = cpool.tile([P, S], f32)
            nc.vector.tensor_scalar(mn[:], ui[:], 1023.0, None,
                                    mybir.AluOpType.min)
            nc.vector.tensor_scalar(idx[:], mn[:], 3.0, None,
                                    mybir.AluOpType.mult)
        else:
            nc.vector.tensor_scalar(idx[:], ui[:], 1023.0, None,
                                    mybir.AluOpType.min)

        spc = S // NCHUNK                        # idx columns per chunk
        for k in range(NCHUNK):
            g = gpool.tile([P, gw], f32, tag=f"g{k}")
            idx_k = idx[:, k * spc:(k + 1) * spc]
            if use_ic:
                d = 1 if planar else F
                nc.gpsimd.indirect_copy(
                    g[:].rearrange("p (n d) -> p n d", d=d),
                    tab[:].rearrange("p (n d) -> p n d", d=d),
                    idx_k, i_know_ap_gather_is_preferred=True)
            else:
                nc.gpsimd.ap_gather(g[:], tab[:], idx_k,
                                    channels=P, num_elems=NBINS,
                                    d=1 if planar else F, num_idxs=IDXS)
            if planar:
                nc.sync.dma_start(out_d.ap()[k, :, :], g[:, :])
            else:
                nc.sync.dma_start(out_d.ap()[:, k, :], g[0:P:16, :])
    nc.compile()
    return nc


def _host_inputs(t, lut):
    planar = GATHER_IMPL.endswith("1")
    if planar:
        lut_rep = np.ascontiguousarray(lut.T[np.arange(P) % 16 % 3])
    else:
        lut_rep = np.ascontiguousarray(
            np.broadcast_to(lut.reshape(-1), (P, NBINS * F)))
    tf = np.ascontiguousarray(np.asarray(t, np.float32)).reshape(-1)
    # SBUF partition 16c+p slot s <- element 2048c + 16s + p of the core chunk
    tperm = (tf.reshape(N_CORES, 8, S, 16).transpose(0, 1, 3, 2)
             .reshape(N_CORES, P, S))
    return tperm, lut_rep


def _host_output(raw):
    """Per-core device output -> (NPC, 3)."""
    if GATHER_IMPL.endswith("1"):
        # raw [NCHUNK, 128, IDXS]; feature f of element (c, 512k+i) is at
        # [k, 16c+f, i]
        r = raw.reshape(NCHUNK, 8, 16, IDXS)[:, :, :F, :]   # k c f i
        return np.ascontiguousarray(r.transpose(1, 0, 3, 2)).reshape(NPC, F)
    # raw [8, NCHUNK, IDXS*F]: (c, k, i*3+f) -> element 2048c + 512k + i
    return raw.reshape(NPC, F)


def kernel(t, W1, b1, W2, b2, W3, b3):
    global LAST_RESULTS
    key = ("nc", GATHER_IMPL)
    if key not in _CACHE:
        _CACHE[key] = _build_nc(GATHER_IMPL)
    nc = _CACHE[key]

    lut = _build_lut(np.asarray(W1, np.float32), np.asarray(b1, np.float32),
                     np.asarray(W2, np.float32), np.asarray(b2, np.float32),
                     np.asarray(W3, np.float32), np.asarray(b3, np.float32))
    tperm, lut_rep = _host_inputs(t, lut)
    in_maps = [{"t": np.ascontiguousarray(tperm[m]), "lut": lut_rep}
               for m in range(N_CORES)]

    res = run_bass_kernel_spmd(nc, in_maps, list(range(N_CORES)), **RUN_KWARGS)
    LAST_RESULTS = res
    outs = [_host_output(res.results[m]["out"]) for m in range(N_CORES)]
    return np.concatenate(outs, axis=0).reshape(B, T, F).astype(np.float32)



# revision 6
# speedup vs baseline: 1.1321x; 1.1321x over previous
"""Trainium2 Bass kernel for the Haar-mask MLP (histogram_binning).

Every Haar interval edge is a multiple of 2^-10, so the reference's masks --
and therefore the entire MLP output -- depend only on u = floor(t * 1024)
(1024 values).  The whole network collapses to a 1024x3 lookup table computed
once on host from the tiny weights.  The device work is evaluating the LUT at
16384 points per core.

GpSimd indirect gathers are SBUF-read-latency bound (~28 ns/index, 57 us per
core), so instead the LUT is evaluated with matmuls over transposed one-hot /
step masks (u = 16*h + l, h in [0,64), l in [0,16)):

  out[f, x] = sum_a [h_x == a] * sum_{l' <= l_x} dLUT[a, l', f]
            = ONES^T @ ( (U_l >= l) * (BDLUT^T @ Mh) )     per column x

  - device computes exact h = floor(64 t), l = floor(1024 t) - 16 h in the
    natural [128p, s] layout (cheap FD), flattens h/l to per-chunk rows via
    SBUF->SBUF DMA, and broadcast-DMAs them across partitions,
  - Mh[64g+a, x] = (h == a) one-hot via tensor_scalar(is_equal) with a
    per-partition AP scalar,
  - mm1: block-diag stationary BDLUT[128, 96] (l-telescoped LUT differences)
    -> D'[96, x] in PSUM; ScalarE evacuates to fp16 SBUF,
  - fused DVE scalar_tensor_tensor: E = (U_l is_ge l_p) * D'  (step mask,
    so the l-selection needs no one-hot; stationary is host-differenced),
  - mm2: ONES[96, 6] -> out[3g+f, x] in PSUM, evacuated fp16, DMA'd out.

Layout: core handles N=16384 elements as 2 chunks (g) x 8192 cols; cols are
processed in 4 quarters of 2048 for DMA/compute pipelining.  fp16 everywhere
on device (integers <= 1024 and LUT deltas are fp16-safe): max rel err vs the
fp32 reference ~6e-4, well under the 2e-2 gate.
"""

from contextlib import ExitStack

import numpy as np

import concourse.tile as tile
from concourse import bacc, mybir
from concourse.bass_utils import run_bass_kernel_spmd

N_CORES = 8
B, T, F = 16, 8192, 3
N = B * T                    # 131072 total elements
NPC = N // N_CORES           # 16384 per neuron core
P = 128
NH, NL = 64, 16              # u = 16*h + l
G = 2                        # chunks per core (64 h-rows each)
CC = NPC // G                # 8192 cols per chunk
NQ = 4                       # col quarters for pipelining
QC = CC // NQ                # 2048 cols per quarter
NB = 2                       # 1024-col blocks per quarter
BC = QC // NB                # 1024
MM = 512                     # moving cols per matmul

GATHER_IMPL = "mm"           # legacy knob (test.py may set it); unused
RUN_KWARGS = {}
LAST_RESULTS = None
_CACHE = {}


def _build_lut(W1, b1, W2, b2, W3, b3):
    """MLP output for each of the 1024 half-interval bins, fp32 math."""
    u = np.arange(1024)
    acc = np.zeros((1024, W1.shape[1]), np.float32)
    for j in range(10):
        k = u >> (10 - j)
        idx = (1 << j) - 1 + k
        sign = np.where((u >> (9 - j)) & 1 == 0, np.float32(1), np.float32(-1))
        acc = acc + sign[:, None] * W1[idx]
    h = np.maximum(acc + b1, np.float32(0))
    h = np.maximum(h @ W2 + b2, np.float32(0))
    return (h @ W3 + b3).astype(np.float32)     # (1024, 3)


def _build_nc():
    nc = bacc.Bacc("TRN2", target_bir_lowering=False, debug=False,
                   enable_asserts=False, num_devices=N_CORES)
    f32 = mybir.dt.float32
    f16 = mybir.dt.float16
    i32 = mybir.dt.int32
    OP = mybir.AluOpType

    t_d = nc.dram_tensor("t", [P, NPC // P], f32, kind="ExternalInput")
    bdlut_d = nc.dram_tensor("bdlut", [P, G * NL * F], f16, kind="ExternalInput")
    ones_d = nc.dram_tensor("ones", [G * NL * F, G * F], f16, kind="ExternalInput")
    aconst_d = nc.dram_tensor("aconst", [P, 1], f32, kind="ExternalInput")
    lconst_d = nc.dram_tensor("lconst", [G * NL * F, 1], f32, kind="ExternalInput")
    out_d = nc.dram_tensor("out", [G * F, CC], f16, kind="ExternalOutput")
    hrow_d = nc.dram_tensor("hrow", [G, CC], f16, kind="Internal")
    lrow_d = nc.dram_tensor("lrow", [G, CC], f16, kind="Internal")

    S = NPC // P             # 128 slots per partition in natural layout
    SQ = S // NQ             # 32 slots per quarter
    R96 = G * NL * F         # 96 rows for U_l / D / E

    with tile.TileContext(nc) as tc, ExitStack() as ctx:
        cpool = ctx.enter_context(tc.tile_pool(name="c", bufs=1))
        qpool = ctx.enter_context(tc.tile_pool(name="q", bufs=1))
        dpool = ctx.enter_context(tc.tile_pool(name="dps", bufs=2, space="PSUM"))
        opool = ctx.enter_context(tc.tile_pool(name="ops", bufs=2, space="PSUM"))
        spool = ctx.enter_context(tc.tile_pool(name="s", bufs=3))

        # ---- constants ----
        bdlut = cpool.tile([P, G * NL * F], f16, tag="bdlut")
        ones = cpool.tile([R96, G * F], f16, tag="ones")
        aconst = cpool.tile([P, 1], f32, tag="aconst")
        lconst = cpool.tile([R96, 1], f32, tag="lconst")
        nc.sync.dma_start(bdlut[:], bdlut_d[:, :])
        nc.sync.dma_start(ones[:], ones_d[:, :])
        nc.sync.dma_start(aconst[:], aconst_d[:, :])
        nc.sync.dma_start(lconst[:], lconst_d[:, :])

        # ---- natural-layout input + u-compute, per half (slots 64H..64H+64) ----
        t_sb = cpool.tile([P, S], f32, tag="t")
        hf = cpool.tile([P, S], f32, tag="hf")
        l16 = cpool.tile([P, S], f16, tag="l16")
        for H in range(2):
            sl = slice(64 * H, 64 * H + 64)
            nc.sync.dma_start(t_sb[:, sl], t_d[:, sl])
            v64 = qpool.tile([P, 64], f32, tag=f"v64_{H}")
            iv = qpool.tile([P, 64], i32, tag=f"iv_{H}")
            fv = qpool.tile([P, 64], f32, tag=f"fv_{H}")
            adj = qpool.tile([P, 64], f32, tag=f"adj_{H}")
            v1k = qpool.tile([P, 64], f32, tag=f"v1k_{H}")
            iv2 = qpool.tile([P, 64], i32, tag=f"iv2_{H}")
            fv2 = qpool.tile([P, 64], f32, tag=f"fv2_{H}")
            adj2 = qpool.tile([P, 64], f32, tag=f"adj2_{H}")
            uf = qpool.tile([P, 64], f32, tag=f"uf_{H}")
            nc.vector.tensor_scalar(v64[:], t_sb[:, sl], 64.0, None, OP.mult)
            nc.vector.tensor_copy(iv[:], v64[:])
            nc.vector.tensor_copy(fv[:], iv[:])
            nc.vector.tensor_tensor(adj[:], fv[:], v64[:], OP.is_gt)
            nc.vector.tensor_tensor(hf[:, sl], fv[:], adj[:], OP.subtract)
            nc.vector.tensor_scalar(v1k[:], t_sb[:, sl], 1024.0, None, OP.mult)
            nc.vector.tensor_copy(iv2[:], v1k[:])
            nc.vector.tensor_copy(fv2[:], iv2[:])
            nc.vector.tensor_tensor(adj2[:], fv2[:], v1k[:], OP.is_gt)
            nc.vector.tensor_tensor(uf[:], fv2[:], adj2[:], OP.subtract)
            # l = u - 16 h  (fp16 out)
            nc.vector.scalar_tensor_tensor(
                l16[:, sl], hf[:, sl], -16.0, uf[:], OP.mult, OP.add)

        # ---- per quarter: flatten rows, broadcast, masks, matmuls ----
        for q in range(NQ):
            ssl = slice(SQ * q, SQ * (q + 1))          # natural slots
            csl = slice(QC * q, QC * (q + 1))          # chunk cols
            for g in range(G):
                psl = slice(64 * g, 64 * g + 64)
                # flatten [64, 32] -> [1, 2048] via DRAM; gpsimd casts f32->f16
                nc.gpsimd.dma_start(hrow_d.ap()[g:g + 1, csl], hf[psl, ssl])
                nc.sync.dma_start(lrow_d.ap()[g:g + 1, csl], l16[psl, ssl])

            uh = qpool.tile([P, QC], f16, tag=f"uh{q}")
            ul = qpool.tile([R96, QC], f16, tag=f"ul{q}")
            nc.sync.dma_start(uh[0:64, :],
                              hrow_d.ap()[0:1, csl].to_broadcast((64, QC)))
            nc.scalar.dma_start(uh[64:128, :],
                                hrow_d.ap()[1:2, csl].to_broadcast((64, QC)))
            nc.sync.dma_start(ul[0:48, :],
                              lrow_d.ap()[0:1, csl].to_broadcast((48, QC)))
            nc.scalar.dma_start(ul[48:96, :],
                                lrow_d.ap()[1:2, csl].to_broadcast((48, QC)))

            mh = qpool.tile([P, QC], f16, tag=f"mh{q}")
            nc.vector.tensor_scalar(mh[:], uh[:], aconst[:, 0:1], None,
                                    OP.is_equal)

            for b in range(NB):
                bsl = slice(BC * b, BC * (b + 1))      # cols within quarter
                dps = dpool.tile([R96, BC], mybir.dt.float32, tag="d")
                for m in range(BC // MM):
                    msl = slice(BC * b + MM * m, BC * b + MM * (m + 1))
                    nc.tensor.matmul(dps[:, MM * m:MM * (m + 1)],
                                     bdlut[:], mh[:, msl],
                                     start=True, stop=True)
                ds = spool.tile([R96, BC], f16, tag="ds")
                nc.scalar.copy(ds[:], dps[:])
                e = spool.tile([R96, BC], f16, tag="e")
                nc.vector.scalar_tensor_tensor(
                    e[:], ul[:, bsl], lconst[:, 0:1], ds[:], OP.is_ge, OP.mult)
                ops = opool.tile([G * F, BC], mybir.dt.float32, tag="o")
                for m in range(BC // MM):
                    nc.tensor.matmul(ops[:, MM * m:MM * (m + 1)],
                                     ones[:], e[:, MM * m:MM * (m + 1)],
                                     start=True, stop=True)
                osb = spool.tile([G * F, BC], f16, tag="osb")
                if b % 2 == 0:
                    nc.scalar.copy(osb[:], ops[:])
                else:
                    nc.vector.tensor_copy(osb[:], ops[:])
                nc.sync.dma_start(out_d.ap()[:, QC * q + BC * b:
                                             QC * q + BC * (b + 1)], osb[:])
    nc.compile()
    return nc


def _host_consts(lut):
    """Block-diag l-telescoped stationary, ones reducer, per-partition consts."""
    lut3 = lut.reshape(NH, NL, F)
    d = lut3.copy()
    d[:, 1:, :] -= lut3[:, :-1, :]              # telescope along l
    d2 = d.reshape(NH, NL * F)                  # col j = 3l + f
    bd = np.zeros((P, G * NL * F), np.float16)
    for g in range(G):
        bd[64 * g:64 * g + 64, 48 * g:48 * g + 48] = d2
    ones = np.zeros((G * NL * F, G * F), np.float16)
    for g in range(G):
        for l in range(NL):
            for f in range(F):
                ones[48 * g + 3 * l + f, 3 * g + f] = 1
    aconst = (np.arange(P) % 64).astype(np.float32).reshape(P, 1)
    lconst = ((np.arange(G * NL * F) % 48) // 3).astype(np.float32).reshape(-1, 1)
    return bd, ones, aconst, lconst


def _host_t(t):
    """Core m natural tile: partition p slot s holds element
    8192*(p//64) + 2048*(s//32) + 32*(p%64) + (s%32) of the core's chunk."""
    tf = np.ascontiguousarray(np.asarray(t, np.float32)).reshape(N_CORES, NPC)
    # index array mapping (p, s) -> element
    p = np.arange(P)[:, None]
    s = np.arange(NPC // P)[None, :]
    e = 8192 * (p // 64) + 2048 * (s // 32) + 32 * (p % 64) + (s % 32)
    return tf[:, e]                              # (N_CORES, 128, 128)


def _host_output(raw):
    """raw [6, 8192] fp16 -> (NPC, 3) fp32; row 3g+f col x = element 8192g+x."""
    r = raw.reshape(G, F, CC).transpose(0, 2, 1).reshape(NPC, F)
    return r.astype(np.float32)


def kernel(t, W1, b1, W2, b2, W3, b3):
    global LAST_RESULTS
    if "nc" not in _CACHE:
        _CACHE["nc"] = _build_nc()
    nc = _CACHE["nc"]

    lut = _build_lut(np.asarray(W1, np.float32), np.asarray(b1, np.float32),
                     np.asarray(W2, np.float32), np.asarray(b2, np.float32),
                     np.asarray(W3, np.float32), np.asarray(b3, np.float32))
    bd, ones, aconst, lconst = _host_consts(lut)
    tperm = _host_t(t)
    in_maps = [{"t": np.ascontiguousarray(tperm[m]), "bdlut": bd,
                "ones": ones, "aconst": aconst, "lconst": lconst}
               for m in range(N_CORES)]

    res = run_bass_kernel_spmd(nc, in_maps, list(range(N_CORES)), **RUN_KWARGS)
    LAST_RESULTS = res
    outs = [_host_output(res.results[m]["out"]) for m in range(N_CORES)]
    return np.concatenate(outs, axis=0).reshape(B, T, F).astype(np.float32)


# revision 9
# speedup vs baseline: 1.4153x; 1.2502x over previous
"""Trainium2 Bass kernel for the Haar-mask MLP (histogram_binning).

Every Haar interval edge is a multiple of 2^-10, so the reference's masks --
and therefore the entire MLP output -- depend only on u = floor(t * 1024)
(1024 values).  The whole network collapses to a 1024x3 lookup table computed
once on host from the tiny weights.  The device work is evaluating the LUT at
16384 points per core.

GpSimd indirect gathers are SBUF-read-latency bound (~28 ns/index, 57 us per
core), so instead the LUT is evaluated with matmuls over transposed one-hot /
step masks (u = 16*h + l, h in [0,64), l in [0,16)):

  out[f, x] = sum_a [h_x == a] * sum_{l' <= l_x} dLUT[a, l', f]
            = ONES^T @ ( (U_l >= l) * (BDLUT^T @ Mh) )     per column x

  - device computes exact h = floor(64 t), l = floor(1024 t) - 16 h in the
    natural [128p, s] layout (cheap FD), flattens h/l to per-chunk rows via
    SBUF->SBUF DMA, and broadcast-DMAs them across partitions,
  - Mh[64g+a, x] = (h == a) one-hot via tensor_scalar(is_equal) with a
    per-partition AP scalar,
  - mm1: block-diag stationary BDLUT[128, 96] (l-telescoped LUT differences)
    -> D'[96, x] in PSUM; ScalarE evacuates to fp16 SBUF,
  - fused DVE scalar_tensor_tensor: E = (U_l is_ge l_p) * D'  (step mask,
    so the l-selection needs no one-hot; stationary is host-differenced),
  - mm2: ONES[96, 6] -> out[3g+f, x] in PSUM, evacuated fp16, DMA'd out.

Layout: core handles N=16384 elements as 2 chunks (g) x 8192 cols; cols are
processed in 4 quarters of 2048 for DMA/compute pipelining.  fp16 everywhere
on device (integers <= 1024 and LUT deltas are fp16-safe): max rel err vs the
fp32 reference ~6e-4, well under the 2e-2 gate.
"""

from contextlib import ExitStack

import numpy as np

import concourse.tile as tile
from concourse import bacc, mybir
from concourse.bass_utils import run_bass_kernel_spmd

N_CORES = 8
B, T, F = 16, 8192, 3
N = B * T                    # 131072 total elements
NPC = N // N_CORES           # 16384 per neuron core
P = 128
NH, NL = 64, 16              # u = 16*h + l
G = 2                        # chunks per core (64 h-rows each)
CC = NPC // G                # 8192 cols per chunk
NQ = 4                       # col quarters for pipelining
QC = CC // NQ                # 2048 cols per quarter
NB = 2                       # 1024-col blocks per quarter
BC = QC // NB                # 1024
MM = 512                     # moving cols per matmul

GATHER_IMPL = "mm"           # legacy knob (test.py may set it); unused
RUN_KWARGS = {}
LAST_RESULTS = None
_CACHE = {}


def _build_lut(W1, b1, W2, b2, W3, b3):
    """MLP output for each of the 1024 half-interval bins, fp32 math."""
    u = np.arange(1024)
    acc = np.zeros((1024, W1.shape[1]), np.float32)
    for j in range(10):
        k = u >> (10 - j)
        idx = (1 << j) - 1 + k
        sign = np.where((u >> (9 - j)) & 1 == 0, np.float32(1), np.float32(-1))
        acc = acc + sign[:, None] * W1[idx]
    h = np.maximum(acc + b1, np.float32(0))
    h = np.maximum(h @ W2 + b2, np.float32(0))
    return (h @ W3 + b3).astype(np.float32)     # (1024, 3)


def _build_nc():
    nc = bacc.Bacc("TRN2", target_bir_lowering=False, debug=False,
                   enable_asserts=False, num_devices=N_CORES)
    f32 = mybir.dt.float32
    f16 = mybir.dt.float16
    i32 = mybir.dt.int32
    OP = mybir.AluOpType

    t_d = nc.dram_tensor("t", [P, NPC // P], f32, kind="ExternalInput")
    bdlut_d = nc.dram_tensor("bdlut", [P, G * NL * F], f16, kind="ExternalInput")
    ones_d = nc.dram_tensor("ones", [G * NL * F, G * F], f16, kind="ExternalInput")
    aconst_d = nc.dram_tensor("aconst", [P, 1], f32, kind="ExternalInput")
    lconst_d = nc.dram_tensor("lconst", [G * NL * F, 1], f32, kind="ExternalInput")
    out_d = nc.dram_tensor("out", [G * F, CC], f16, kind="ExternalOutput")
    hrow_d = nc.dram_tensor("hrow", [G, CC], f16, kind="Internal")
    lrow_d = nc.dram_tensor("lrow", [G, CC], f16, kind="Internal")

    S = NPC // P             # 128 slots per partition in natural layout
    SQ = S // NQ             # 32 slots per quarter
    HC = CC // 2             # 4096 cols per half
    R96 = G * NL * F         # 96 rows for U_l / D / E

    with tile.TileContext(nc) as tc, ExitStack() as ctx:
        cpool = ctx.enter_context(tc.tile_pool(name="c", bufs=1))
        qpool = ctx.enter_context(tc.tile_pool(name="q", bufs=1))
        dpool = ctx.enter_context(tc.tile_pool(name="dps", bufs=2, space="PSUM"))
        opool = ctx.enter_context(tc.tile_pool(name="ops", bufs=2, space="PSUM"))
        spool = ctx.enter_context(tc.tile_pool(name="s", bufs=3))

        # ---- constants ----
        bdlut = cpool.tile([P, G * NL * F], f16, tag="bdlut")
        ones = cpool.tile([R96, G * F], f16, tag="ones")
        aconst = cpool.tile([P, 1], f32, tag="aconst")
        lconst = cpool.tile([R96, 1], f32, tag="lconst")
        nc.sync.dma_start(bdlut[:], bdlut_d[:, :])
        nc.sync.dma_start(ones[:], ones_d[:, :])
        nc.sync.dma_start(aconst[:], aconst_d[:, :])
        nc.sync.dma_start(lconst[:], lconst_d[:, :])

        # ---- natural-layout input + u-compute, per half (slots 64H..64H+64) ----
        t_sb = cpool.tile([P, S], f32, tag="t")
        h16 = cpool.tile([P, S], f16, tag="h16")
        l16 = cpool.tile([P, S], f16, tag="l16")
        uh = cpool.tile([P, CC], f16, tag="uh")
        ul = cpool.tile([R96, CC], f16, tag="ul")
        for H in range(2):
            sl = slice(64 * H, 64 * H + 64)
            csl = slice(HC * H, HC * (H + 1))          # chunk cols of half
            nc.sync.dma_start(t_sb[:, sl], t_d[:, sl])
            v64 = qpool.tile([P, 64], f32, tag=f"v64_{H}")
            iv = qpool.tile([P, 64], i32, tag=f"iv_{H}")
            fv = qpool.tile([P, 64], f32, tag=f"fv_{H}")
            adj = qpool.tile([P, 64], f32, tag=f"adj_{H}")
            hf = qpool.tile([P, 64], f32, tag=f"hf_{H}")
            v1k = qpool.tile([P, 64], f32, tag=f"v1k_{H}")
            iv2 = qpool.tile([P, 64], i32, tag=f"iv2_{H}")
            fv2 = qpool.tile([P, 64], f32, tag=f"fv2_{H}")
            adj2 = qpool.tile([P, 64], f32, tag=f"adj2_{H}")
            uf = qpool.tile([P, 64], f32, tag=f"uf_{H}")
            nc.vector.tensor_scalar(v64[:], t_sb[:, sl], 64.0, None, OP.mult)
            nc.vector.tensor_copy(iv[:], v64[:])
            nc.vector.tensor_copy(fv[:], iv[:])
            nc.vector.tensor_tensor(adj[:], fv[:], v64[:], OP.is_gt)
            nc.vector.tensor_tensor(hf[:], fv[:], adj[:], OP.subtract)
            nc.vector.tensor_scalar(v1k[:], t_sb[:, sl], 1024.0, None, OP.mult)
            nc.vector.tensor_copy(iv2[:], v1k[:])
            nc.vector.tensor_copy(fv2[:], iv2[:])
            nc.vector.tensor_tensor(adj2[:], fv2[:], v1k[:], OP.is_gt)
            nc.vector.tensor_tensor(uf[:], fv2[:], adj2[:], OP.subtract)
            nc.vector.tensor_copy(h16[:, sl], hf[:])
            # l = u - 16 h  (fp16 out)
            nc.vector.scalar_tensor_tensor(
                l16[:, sl], hf[:], -16.0, uf[:], OP.mult, OP.add)

            # flatten halves to per-chunk DRAM rows (all fp16, HWDGE)
            for g in range(G):
                psl = slice(64 * g, 64 * g + 64)
                eng = nc.sync if g == 0 else nc.scalar
                eng.dma_start(hrow_d.ap()[g:g + 1, csl], h16[psl, sl])
                eng.dma_start(lrow_d.ap()[g:g + 1, csl], l16[psl, sl])
            # broadcast rows across partitions (DRAM -> SBUF)
            nc.sync.dma_start(uh[0:64, csl],
                              hrow_d.ap()[0:1, csl].to_broadcast((64, HC)))
            nc.scalar.dma_start(uh[64:128, csl],
                                hrow_d.ap()[1:2, csl].to_broadcast((64, HC)))
            nc.sync.dma_start(ul[0:48, csl],
                              lrow_d.ap()[0:1, csl].to_broadcast((48, HC)))
            nc.scalar.dma_start(ul[48:96, csl],
                                lrow_d.ap()[1:2, csl].to_broadcast((48, HC)))

        # ---- per quarter: h one-hot, matmuls, fused step-mask multiply ----
        mh = cpool.tile([P, CC], f16, tag="mh")
        for q in range(NQ):
            qsl = slice(QC * q, QC * (q + 1))
            nc.vector.tensor_scalar(mh[:, qsl], uh[:, qsl], aconst[:, 0:1],
                                    None, OP.is_equal)
            dtiles = []
            for b in range(NB):
                dps = dpool.tile([R96, BC], mybir.dt.float32, tag="d")
                dtiles.append(dps)
                for m in range(BC // MM):
                    msl = slice(QC * q + BC * b + MM * m,
                                QC * q + BC * b + MM * (m + 1))
                    nc.tensor.matmul(dps[:, MM * m:MM * (m + 1)],
                                     bdlut[:], mh[:, msl],
                                     start=True, stop=True)
            etiles = []
            for b in range(NB):
                bsl = slice(QC * q + BC * b, QC * q + BC * (b + 1))
                e = spool.tile([R96, BC], f16, tag="e")
                etiles.append(e)
                # fused step mask * D, reading D straight from PSUM
                nc.vector.scalar_tensor_tensor(
                    e[:], ul[:, bsl], lconst[:, 0:1], dtiles[b][:],
                    OP.is_ge, OP.mult)
            otiles = []
            for b in range(NB):
                ops = opool.tile([G * F, BC], mybir.dt.float32, tag="o")
                otiles.append(ops)
                for m in range(BC // MM):
                    nc.tensor.matmul(ops[:, MM * m:MM * (m + 1)],
                                     ones[:], etiles[b][:, MM * m:MM * (m + 1)],
                                     start=True, stop=True)
            for b in range(NB):
                osb = spool.tile([G * F, BC], f16, tag="osb")
                nc.scalar.copy(osb[:], otiles[b][:])
                nc.sync.dma_start(out_d.ap()[:, QC * q + BC * b:
                                             QC * q + BC * (b + 1)], osb[:])
    nc.compile()
    return nc


def _host_consts(lut):
    """Block-diag l-telescoped stationary, ones reducer, per-partition consts."""
    lut3 = lut.reshape(NH, NL, F)
    d = lut3.copy()
    d[:, 1:, :] -= lut3[:, :-1, :]              # telescope along l
    d2 = d.reshape(NH, NL * F)                  # col j = 3l + f
    bd = np.zeros((P, G * NL * F), np.float16)
    for g in range(G):
        bd[64 * g:64 * g + 64, 48 * g:48 * g + 48] = d2
    ones = np.zeros((G * NL * F, G * F), np.float16)
    for g in range(G):
        for l in range(NL):
            for f in range(F):
                ones[48 * g + 3 * l + f, 3 * g + f] = 1
    aconst = (np.arange(P) % 64).astype(np.float32).reshape(P, 1)
    lconst = ((np.arange(G * NL * F) % 48) // 3).astype(np.float32).reshape(-1, 1)
    return bd, ones, aconst, lconst


def _host_t(t):
    """Core m natural tile: partition p slot s holds element
    8192*(p//64) + 2048*(s//32) + 32*(p%64) + (s%32) of the core's chunk."""
    tf = np.ascontiguousarray(np.asarray(t, np.float32)).reshape(N_CORES, NPC)
    # index array mapping (p, s) -> element
    p = np.arange(P)[:, None]
    s = np.arange(NPC // P)[None, :]
    e = 8192 * (p // 64) + 4096 * (s // 64) + 64 * (p % 64) + (s % 64)
    return tf[:, e]                              # (N_CORES, 128, 128)


def _host_output(raw):
    """raw [6, 8192] fp16 -> (NPC, 3) fp32; row 3g+f col x = element 8192g+x."""
    r = raw.reshape(G, F, CC).transpose(0, 2, 1).reshape(NPC, F)
    return r.astype(np.float32)


def kernel(t, W1, b1, W2, b2, W3, b3):
    global LAST_RESULTS
    if "nc" not in _CACHE:
        _CACHE["nc"] = _build_nc()
    nc = _CACHE["nc"]

    lut = _build_lut(np.asarray(W1, np.float32), np.asarray(b1, np.float32),
                     np.asarray(W2, np.float32), np.asarray(b2, np.float32),
                     np.asarray(W3, np.float32), np.asarray(b3, np.float32))
    bd, ones, aconst, lconst = _host_consts(lut)
    tperm = _host_t(t)
    in_maps = [{"t": np.ascontiguousarray(tperm[m]), "bdlut": bd,
                "ones": ones, "aconst": aconst, "lconst": lconst}
               for m in range(N_CORES)]

    res = run_bass_kernel_spmd(nc, in_maps, list(range(N_CORES)), **RUN_KWARGS)
    LAST_RESULTS = res
    outs = [_host_output(res.results[m]["out"]) for m in range(N_CORES)]
    return np.concatenate(outs, axis=0).reshape(B, T, F).astype(np.float32)
